# revision 1
# baseline (speedup 1.0000x reference)
"""Multi-head attention (B=4, S=2048, D=1024, H=16) on 8 TRN2 NeuronCores.

Sharding: core c handles batch b = c//2 and query-half qh = c%2 (1024 query
rows), with K/V projection for its batch replicated across the 2 cores that
share the batch. Zero inter-core communication; host just slices inputs and
concatenates outputs.

Per-core dataflow (all matmuls float32r unless noted):
  1. PE-transpose inputs to X^T layout ([d_model on partitions, seq free]).
  2. Projections: Q^T/K^T = W^T chunks @ X^T  (evicted to bf16, +bias),
     V = X^T-chunks(stationary) @ Wv (normal [s, dv] layout, f32r).
  3. Per head-pair, per q-tile(512): scores^T = K_h^T.T @ Q_h^T (bf16 matmul,
     2 heads row-packed in the PE array), exp via ScalarE (scale=1/32) to
     f32r, PV col-packed (2 heads), softmax sums via ones-matmul (M=1),
     normalize O^T with GPSIMD-broadcast reciprocals (+bv).
  4. Final: out = O^T-chunks.T @ Wo + bo (bo added via a K=1 ones matmul).
"""

import numpy as np

import concourse.bacc as bacc
import concourse.mybir as mybir
import concourse.tile as tile
from concourse import bass_utils
from concourse.masks import make_identity

F32 = mybir.dt.float32
F32R = mybir.dt.float32r
BF16 = mybir.dt.bfloat16
EXP = mybir.ActivationFunctionType.Exp
COPY = mybir.ActivationFunctionType.Copy

B, S, D, H = 4, 2048, 1024, 16
SQ = 1024          # query rows per core
P = 128
MC = D // P        # 8 m-chunks (contraction of projections)
DKC = D // P       # 8 dk-chunks
KC = S // P        # 16 key chunks
SCALE = 1.0 / 32.0  # 1/sqrt(D_K)
N_CORES = 8

_CACHED_NC = None


def build_nc():
    nc = bacc.Bacc("TRN2", target_bir_lowering=False, debug=False,
                   num_devices=N_CORES)
    q_in = nc.dram_tensor("q_in", [SQ, D], F32, kind="ExternalInput")
    k_in = nc.dram_tensor("k_in", [S, D], F32, kind="ExternalInput")
    v_in = nc.dram_tensor("v_in", [S, D], F32, kind="ExternalInput")
    wq_d = nc.dram_tensor("wq", [D, D], F32, kind="ExternalInput")
    wk_d = nc.dram_tensor("wk", [D, D], F32, kind="ExternalInput")
    wv_d = nc.dram_tensor("wv", [D, D], F32, kind="ExternalInput")
    wo_d = nc.dram_tensor("wo", [D, D], F32, kind="ExternalInput")
    bq_d = nc.dram_tensor("bq", [D], F32, kind="ExternalInput")
    bk_d = nc.dram_tensor("bk", [D], F32, kind="ExternalInput")
    bv_d = nc.dram_tensor("bv", [D], F32, kind="ExternalInput")
    bo_d = nc.dram_tensor("bo", [D], F32, kind="ExternalInput")
    out_d = nc.dram_tensor("out", [SQ, D], F32, kind="ExternalOutput")

    with tile.TileContext(nc) as tc:
        with tc.tile_pool(name="const", bufs=1) as constp:
            ident = constp.tile([P, P], F32)
            make_identity(nc, ident[:])
            ones_f = constp.tile([P, 1], F32)
            nc.vector.memset(ones_f[:], 1.0)
            ones_col = constp.tile([P, 1], F32R)
            nc.vector.tensor_copy(ones_col[:], ones_f[:])
            onesr_f = constp.tile([1, P], F32)
            nc.vector.memset(onesr_f[:], 1.0)
            ones_row = constp.tile([1, P], F32R)
            nc.vector.tensor_copy(ones_row[:], onesr_f[:])
            bq_t = constp.tile([P, MC], F32)
            nc.sync.dma_start(bq_t[:], bq_d.ap().rearrange("(c p) -> p c", p=P))
            bk_t = constp.tile([P, MC], F32)
            nc.sync.dma_start(bk_t[:], bk_d.ap().rearrange("(c p) -> p c", p=P))
            bv_t = constp.tile([P, MC], F32)
            nc.sync.dma_start(bv_t[:], bv_d.ap().rearrange("(c p) -> p c", p=P))
            bo_f = constp.tile([1, D], F32)
            nc.sync.dma_start(bo_f[:], bo_d.ap().unsqueeze(0))
            bo_t = constp.tile([1, D], F32R)
            nc.vector.tensor_copy(bo_t[:], bo_f[:])

            _build_body(nc, tc, q_in, k_in, v_in, wq_d, wk_d, wv_d, wo_d,
                        bq_t, bk_t, bv_t, bo_t, ident, ones_col, ones_row,
                        out_d)
    nc.compile()
    return nc


def _load_w(nc, wpool, stg, w_d, tag):
    """DMA weight matrix row-chunks and round to f32r. Returns 8 tiles
    [128, D] (f32r), tile mm = rows [128*mm, 128*mm+128)."""
    tiles = []
    for mm in range(MC):
        raw = stg.tile([P, D], F32, tag="wraw")
        nc.sync.dma_start(raw[:], w_d.ap()[mm * P:(mm + 1) * P, :])
        t = wpool.tile([P, D], F32R, tag=f"{tag}{mm}", name=f"wt_{tag}{mm}")
        nc.vector.tensor_copy(t[:], raw[:])
        tiles.append(t)
    return tiles


def _transpose_groups(nc, x_d, n_rows, stg, ps_t, ident, evict):
    """PE-transpose x_d [n_rows, D] in groups of 4 row-chunks.

    For each group g and m-chunk mm, produces a [128, 512] transposed block
    (partitions = m, free = the group's 4x128 seq rows) in PSUM and calls
    evict(mm, g, psum_slice) to store it."""
    ngroups = n_rows // (4 * P)
    for g in range(ngroups):
        rows = []
        for j in range(4):
            r = g * 4 + j
            t = stg.tile([P, D], F32, tag="xin", bufs=6)
            nc.sync.dma_start(t[:], x_d.ap()[r * P:(r + 1) * P, :])
            rows.append(t)
        for mm in range(MC):
            pst = ps_t.tile([P, 512], F32, tag="pst")
            for j in range(4):
                nc.tensor.transpose(
                    pst[:, j * P:(j + 1) * P],
                    rows[j][:, mm * P:(mm + 1) * P], ident[:])
            evict(mm, g, pst)


def _normalize_pair(nc, OT, rp, bcp, bv_t, qs, pair, pv1, pv2):
    """Softmax-normalize both heads of a pair from fused PV psums
    (row 64 = sums) into OT; odd head partition-shifted via DMA.

    The psum is first evicted wholesale to SBUF with one DVE copy so the
    PSUM bank frees fast (keeps the PV accumulation pipeline moving)."""
    F32_, F32R_ = F32, F32R
    for hh, pvp in ((0, pv1), (1, pv2)):
        rb = rp.tile([P, 512], F32_, tag="rb", bufs=1, name="rb")
        nc.vector.tensor_copy(rb[64:65, :], pvp[64:65, :])
        r0 = rp.tile([1, 512], F32_, tag="r0", bufs=2, name="r0")
        nc.gpsimd.tensor_copy(r0[:], rb[64:65, :])
        rr = rp.tile([1, 512], F32_, tag="rr", bufs=2, name="rr")
        nc.vector.reciprocal_approx_fast(rr[:], r0[:])
        bc = bcp.tile([64, 512], F32_, tag="bc", name="bc")
        nc.gpsimd.partition_broadcast(bc[:], rr[:])
        if hh == 0:
            osl = OT[pair][0:64, qs]
            nc.vector.tensor_mul(osl, pvp[0:64, :], bc[:])
            nc.vector.tensor_scalar_add(osl, osl, bv_t[0:64, pair:pair + 1])
        else:
            tmp = bcp.tile([64, 512], F32R_, tag="tmp", bufs=1, name="tmp")
            nc.vector.tensor_mul(tmp[:], pvp[0:64, :], bc[:])
            osl = OT[pair][64:128, qs]
            nc.sync.dma_start(osl, tmp[:])
            nc.vector.tensor_scalar_add(osl, osl, bv_t[64:128, pair:pair + 1])


def _build_body(nc, tc, q_in, k_in, v_in, wq_d, wk_d, wv_d, wo_d,
                bq_t, bk_t, bv_t, bo_t, ident, ones_col, ones_row, out_d):
    # ---------------- persistent pools (LIFO stack) ----------------
    with tc.tile_pool(name="qtp", bufs=1) as qtp:
        QT = [qtp.tile([P, SQ], BF16, tag=f"qt{i}", name=f"qt{i}") for i in range(DKC)]

        # ---- stage Q ----
        with (
            tc.tile_pool(name="xtq", bufs=1) as xtp,
            tc.tile_pool(name="wq", bufs=1) as wpool,
            tc.tile_pool(name="stgq", bufs=2) as stg,
            tc.tile_pool(name="psq_t", bufs=2, space="PSUM") as ps_t,
            tc.tile_pool(name="psq_p", bufs=2, space="PSUM") as ps_p,
        ):
            xqT = [xtp.tile([P, SQ], F32R, tag=f"xt{i}", name=f"xqt{i}") for i in range(MC)]
            wq_t = _load_w(nc, wpool, stg, wq_d, "w")

            def evq(mm, g, pst):
                nc.scalar.activation(
                    xqT[mm][:, g * 512:(g + 1) * 512], pst[:], COPY)
            _transpose_groups(nc, q_in, SQ, stg, ps_t, ident, evq)

            for dk in range(DKC):
                for nh in range(SQ // 512):
                    ps = ps_p.tile([P, 512], F32, tag="pp")
                    for mm in range(MC):
                        nc.tensor.matmul(
                            ps[:], wq_t[mm][:, dk * P:(dk + 1) * P],
                            xqT[mm][:, nh * 512:(nh + 1) * 512],
                            start=(mm == 0), stop=(mm == MC - 1))
                    nc.vector.tensor_scalar_add(
                        QT[dk][:, nh * 512:(nh + 1) * 512], ps[:],
                        bq_t[:, dk:dk + 1])

        with tc.tile_pool(name="ktp", bufs=1) as ktp:
            KT = [ktp.tile([P, S], BF16, tag=f"kt{i}", name=f"kt{i}") for i in range(DKC)]

            # ---- stage K ----
            with (
                tc.tile_pool(name="xtk", bufs=1) as xtp,
                tc.tile_pool(name="wk", bufs=1) as wpool,
                tc.tile_pool(name="stgk", bufs=2) as stg,
                tc.tile_pool(name="psk_t", bufs=2, space="PSUM") as ps_t,
                tc.tile_pool(name="psk_p", bufs=2, space="PSUM") as ps_p,
            ):
                xkT = [xtp.tile([P, S], F32R, tag=f"xt{i}", name=f"xkt{i}") for i in range(MC)]
                wk_t = _load_w(nc, wpool, stg, wk_d, "w")

                def evk(mm, g, pst):
                    nc.scalar.activation(
                        xkT[mm][:, g * 512:(g + 1) * 512], pst[:], COPY)
                _transpose_groups(nc, k_in, S, stg, ps_t, ident, evk)

                for dk in range(DKC):
                    for nh in range(S // 512):
                        ps = ps_p.tile([P, 512], F32, tag="pp")
                        for mm in range(MC):
                            nc.tensor.matmul(
                                ps[:], wk_t[mm][:, dk * P:(dk + 1) * P],
                                xkT[mm][:, nh * 512:(nh + 1) * 512],
                                start=(mm == 0), stop=(mm == MC - 1))
                        nc.vector.tensor_scalar_add(
                            KT[dk][:, nh * 512:(nh + 1) * 512], ps[:],
                            bk_t[:, dk:dk + 1])

            with tc.tile_pool(name="vp", bufs=1) as vp:
                DEXT = H * 65  # V_ext: 65 cols per head (64 V + ones)
                V = [vp.tile([P, DEXT], F32R, tag=f"v{i}", name=f"v{i}")
                     for i in range(KC)]

                # ---- stage V ----
                with (
                    tc.tile_pool(name="vtt", bufs=1) as vtt,
                    tc.tile_pool(name="wv", bufs=1) as wpool,
                    tc.tile_pool(name="stgv", bufs=2) as stg,
                    tc.tile_pool(name="psv_t", bufs=2, space="PSUM") as ps_t,
                    tc.tile_pool(name="psv_p", bufs=2, space="PSUM") as ps_p,
                ):
                    wv_t = _load_w(nc, wpool, stg, wv_d, "w")
                    valT = [vtt.tile([P, 512], F32R, tag=f"vt{i}", name=f"vt{i}")
                            for i in range(MC)]
                    ones16 = vtt.tile([P, H], F32, name="ones16")
                    nc.vector.memset(ones16[:], 1.0)

                    ngroups = S // (4 * P)
                    for g in range(ngroups):
                        rows = []
                        for j in range(4):
                            r = g * 4 + j
                            t = stg.tile([P, D], F32, tag="xin", bufs=6)
                            nc.sync.dma_start(t[:], v_in.ap()[r * P:(r + 1) * P, :])
                            rows.append(t)
                        for mm in range(MC):
                            pst = ps_t.tile([P, 512], F32, tag="pst")
                            for j in range(4):
                                nc.tensor.transpose(
                                    pst[:, j * P:(j + 1) * P],
                                    rows[j][:, mm * P:(mm + 1) * P], ident[:])
                            nc.scalar.activation(valT[mm][:], pst[:], COPY)
                        for j in range(4):
                            sc = g * 4 + j
                            vx = V[sc].rearrange("p (h c) -> p h c", c=65)
                            nc.vector.tensor_copy(
                                vx[:, :, 64:65],
                                ones16[:].rearrange("p (h c) -> p h c", c=1))
                            for nh in range(2):
                                ps = ps_p.tile([P, 512], F32, tag="pp")
                                for mm in range(MC):
                                    nc.tensor.matmul(
                                        ps[:], valT[mm][:, j * P:(j + 1) * P],
                                        wv_t[mm][:, nh * 512:(nh + 1) * 512],
                                        start=(mm == 0), stop=(mm == MC - 1))
                                nc.vector.tensor_copy(
                                    vx[:, 8 * nh:8 * nh + 8, 0:64],
                                    ps[:].rearrange("p (h c) -> p h c", c=64))

                with tc.tile_pool(name="otp", bufs=1) as otp:
                    OT = [otp.tile([P, SQ], F32R, tag=f"ot{i}", name=f"ot{i}")
                          for i in range(DKC)]

                    # ---- attention + final ----
                    with (
                        tc.tile_pool(name="ep", bufs=4) as ep,
                        tc.tile_pool(name="bcp", bufs=2) as bcp,
                        tc.tile_pool(name="rp", bufs=3) as rp,
                        tc.tile_pool(name="ps_sc", bufs=2, space="PSUM") as ps_sc,
                        tc.tile_pool(name="ps_pv", bufs=4, space="PSUM") as ps_pv,
                        tc.tile_pool(name="wo", bufs=1) as wop,
                        tc.tile_pool(name="fin", bufs=2) as finp,
                    ):
                        for qt in range(SQ // 512):
                            qs = slice(qt * 512, (qt + 1) * 512)
                            for pair in range(H // 2):
                                pv1 = ps_pv.tile([P, 512], F32, tag="pv")
                                pv2 = ps_pv.tile([P, 512], F32, tag="pv")
                                c1 = (2 * pair) * 65
                                c2 = (2 * pair + 1) * 65
                                for k2 in range(KC // 2):
                                    ka = slice(2 * k2 * P, (2 * k2 + 1) * P)
                                    kb = slice((2 * k2 + 1) * P,
                                               (2 * k2 + 2) * P)
                                    s1 = ps_sc.tile([P, 1024], F32, tag="sc")
                                    s2 = ps_sc.tile([P, 1024], F32, tag="sc")
                                    nc.tensor.matmul(
                                        s1[:, 0:512], KT[pair][0:64, ka],
                                        QT[pair][0:64, qs],
                                        start=True, stop=True,
                                        tile_position=(0, 0))
                                    nc.tensor.matmul(
                                        s2[:, 0:512], KT[pair][64:128, ka],
                                        QT[pair][64:128, qs],
                                        start=True, stop=True,
                                        tile_position=(64, 0))
                                    nc.tensor.matmul(
                                        s1[:, 512:1024], KT[pair][0:64, kb],
                                        QT[pair][0:64, qs],
                                        start=True, stop=True,
                                        tile_position=(0, 0))
                                    nc.tensor.matmul(
                                        s2[:, 512:1024], KT[pair][64:128, kb],
                                        QT[pair][64:128, qs],
                                        start=True, stop=True,
                                        tile_position=(64, 0))
                                    e1 = ep.tile([P, 1024], F32R, tag="e")
                                    e2 = ep.tile([P, 1024], F32R, tag="e")
                                    nc.scalar.activation(e1[:], s1[:], EXP,
                                                         scale=SCALE)
                                    nc.scalar.activation(e2[:], s2[:], EXP,
                                                         scale=SCALE)
                                    first = k2 == 0
                                    last = k2 == KC // 2 - 1
                                    nc.tensor.matmul(
                                        pv1[0:65, :],
                                        V[2 * k2][:, c1:c1 + 65],
                                        e1[:, 0:512], start=first,
                                        stop=False)
                                    nc.tensor.matmul(
                                        pv2[0:65, :],
                                        V[2 * k2][:, c2:c2 + 65],
                                        e2[:, 0:512], start=first,
                                        stop=False)
                                    nc.tensor.matmul(
                                        pv1[0:65, :],
                                        V[2 * k2 + 1][:, c1:c1 + 65],
                                        e1[:, 512:1024], start=False,
                                        stop=last)
                                    nc.tensor.matmul(
                                        pv2[0:65, :],
                                        V[2 * k2 + 1][:, c2:c2 + 65],
                                        e2[:, 512:1024], start=False,
                                        stop=last)
                                _normalize_pair(nc, OT, rp, bcp, bv_t,
                                                qs, pair, pv1, pv2)

                            # final projection for this q-tile's s-chunks
                            for nh in range(2):
                                ns = slice(nh * 512, (nh + 1) * 512)
                                wo_h = []
                                for dk in range(DKC):
                                    raw = finp.tile([P, 512], F32, tag="wraw", bufs=1)
                                    nc.sync.dma_start(
                                        raw[:], wo_d.ap()[dk * P:(dk + 1) * P, ns])
                                    wt = wop.tile([P, 512], F32R,
                                                  tag=f"woh{dk}", name=f"woh{dk}")
                                    nc.vector.tensor_copy(wt[:], raw[:])
                                    wo_h.append(wt)
                                for sc in range(qt * 4, (qt + 1) * 4):
                                    ss = slice(sc * P, (sc + 1) * P)
                                    fps = ps_pv.tile([P, 512], F32, tag="pv")
                                    for dk in range(DKC):
                                        nc.tensor.matmul(
                                            fps[:], OT[dk][:, ss],
                                            wo_h[dk][:],
                                            start=(dk == 0), stop=False)
                                    nc.tensor.matmul(
                                        fps[:], ones_row[:], bo_t[:, ns],
                                        start=False, stop=True)
                                    ob = finp.tile([P, 512], F32, tag="ob", bufs=1)
                                    nc.vector.tensor_copy(ob[:], fps[:])
                                    nc.sync.dma_start(out_d.ap()[ss, ns], ob[:])


def get_nc():
    global _CACHED_NC
    if _CACHED_NC is None:
        _CACHED_NC = build_nc()
    return _CACHED_NC


def run(inputs, **kwargs):
    """Run on 8 cores; returns (full_output, BassKernelResults)."""
    nc = get_nc()
    queries = np.ascontiguousarray(np.asarray(inputs["queries"], np.float32))
    keys = np.ascontiguousarray(np.asarray(inputs["keys"], np.float32))
    values = np.ascontiguousarray(np.asarray(inputs["values"], np.float32))
    base = {
        "wq": np.ascontiguousarray(np.asarray(inputs["Wq"], np.float32)),
        "wk": np.ascontiguousarray(np.asarray(inputs["Wk"], np.float32)),
        "wv": np.ascontiguousarray(np.asarray(inputs["Wv"], np.float32)),
        "wo": np.ascontiguousarray(np.asarray(inputs["Wo"], np.float32)),
        "bq": np.ascontiguousarray(np.asarray(inputs["bq"], np.float32)),
        "bk": np.ascontiguousarray(np.asarray(inputs["bk"], np.float32)),
        "bv": np.ascontiguousarray(np.asarray(inputs["bv"], np.float32)),
        "bo": np.ascontiguousarray(np.asarray(inputs["bo"], np.float32)),
    }
    in_maps = []
    for c in range(N_CORES):
        b, qh = c // 2, c % 2
        m = dict(base)
        m["q_in"] = np.ascontiguousarray(queries[b, qh * SQ:(qh + 1) * SQ])
        m["k_in"] = keys[b]
        m["v_in"] = values[b]
        in_maps.append(m)
    res = bass_utils.run_bass_kernel_spmd(
        nc, in_maps, core_ids=list(range(N_CORES)), **kwargs)
    out = np.empty((B, S, D), np.float32)
    for c in range(N_CORES):
        b, qh = c // 2, c % 2
        out[b, qh * SQ:(qh + 1) * SQ] = res.results[c]["out"]
    return out, res


def kernel(**inputs):
    out, _ = run(inputs)
    return out


if __name__ == "__main__":
    rng = np.random.default_rng(0)
    ins = {
        "queries": rng.standard_normal((B, S, D), dtype=np.float32),
        "keys": rng.standard_normal((B, S, D), dtype=np.float32),
        "values": rng.standard_normal((B, S, D), dtype=np.float32),
        "Wq": (rng.standard_normal((D, D), dtype=np.float32) / 32),
        "bq": np.zeros(D, np.float32),
        "Wk": (rng.standard_normal((D, D), dtype=np.float32) / 32),
        "bk": np.zeros(D, np.float32),
        "Wv": (rng.standard_normal((D, D), dtype=np.float32) / 32),
        "bv": np.zeros(D, np.float32),
        "Wo": (rng.standard_normal((D, D), dtype=np.float32) / 32),
        "bo": np.zeros(D, np.float32),
    }
    out = kernel(**ins)
    print("out", out.shape, out.dtype, np.abs(out).mean())



# revision 4
# speedup vs baseline: 1.0936x; 1.0936x over previous
"""Multi-head attention (B=4, S=2048, D=1024, H=16) on 8 TRN2 NeuronCores.

Sharding: core c handles batch b = c//2 and query-half qh = c%2 (1024 query
rows), with K/V projection for its batch replicated across the 2 cores that
share the batch. Zero inter-core communication; host just slices inputs and
concatenates outputs.

Per-core dataflow (all matmuls bf16, accumulated in f32 PSUM):
  1. Inputs are DVE-cast to bf16, PE-transposed to X^T layout.
  2. Projections: Q^T/K^T = W^T chunks @ X^T (bf16, bias fused in the DVE
     eviction); V = X^T-chunks(stationary) @ Wv with a ones column per head
     (softmax denominators ride the PV matmul).
  3. Attention per head-pair, per q-tile(512): scores^T via row-packed
     bf16 matmuls (2 heads in PE quadrants), exp on ScalarE (the ONLY
     ScalarE work - it is the critical resource) into bf16, PV col-serial,
     normalization via DVE/GPSIMD reciprocal-broadcast.
  4. Final: out = O^T-chunks.T @ Wo (+bo via K=1 ones matmul).

Scheduling: projection chunks for head-pair p+1, the whole V pipeline and
the Wo load are emitted as "filler" interleaved into the attention slots of
earlier pairs, so the PE and ScalarE streams overlap end-to-end instead of
running in serial phases (keeps the PE HAM clock-gate warm).
"""

from collections import deque

import numpy as np

import concourse.bacc as bacc
import concourse.mybir as mybir
import concourse.tile as tile
from concourse import bass_utils
from concourse.masks import make_identity

F32 = mybir.dt.float32
F32R = mybir.dt.float32r
BF16 = mybir.dt.bfloat16
EXP = mybir.ActivationFunctionType.Exp

B, S, D, H = 4, 2048, 1024, 16
SQ = 1024          # query rows per core
P = 128
MC = D // P        # 8 m-chunks (contraction of projections)
DKC = D // P       # 8 dk-chunks == head pairs
KC = S // P        # 16 key chunks
SCALE = 1.0 / 32.0  # 1/sqrt(D_K)
N_CORES = 8

_CACHED_NC = None


def build_nc():
    nc = bacc.Bacc("TRN2", target_bir_lowering=False, debug=False,
                   num_devices=N_CORES)
    q_in = nc.dram_tensor("q_in", [SQ, D], F32, kind="ExternalInput")
    k_in = nc.dram_tensor("k_in", [S, D], F32, kind="ExternalInput")
    v_in = nc.dram_tensor("v_in", [S, D], F32, kind="ExternalInput")
    wq_d = nc.dram_tensor("wq", [D, D], F32, kind="ExternalInput")
    wk_d = nc.dram_tensor("wk", [D, D], F32, kind="ExternalInput")
    wv_d = nc.dram_tensor("wv", [D, D], F32, kind="ExternalInput")
    wo_d = nc.dram_tensor("wo", [D, D], F32, kind="ExternalInput")
    bq_d = nc.dram_tensor("bq", [D], F32, kind="ExternalInput")
    bk_d = nc.dram_tensor("bk", [D], F32, kind="ExternalInput")
    bv_d = nc.dram_tensor("bv", [D], F32, kind="ExternalInput")
    bo_d = nc.dram_tensor("bo", [D], F32, kind="ExternalInput")
    out_d = nc.dram_tensor("out", [SQ, D], F32, kind="ExternalOutput")

    with tile.TileContext(nc) as tc:
        _build_body(nc, tc, q_in, k_in, v_in, wq_d, wk_d, wv_d, wo_d,
                    bq_d, bk_d, bv_d, bo_d, out_d)
    nc.compile()
    return nc


def _build_body(nc, tc, q_in, k_in, v_in, wq_d, wk_d, wv_d, wo_d,
                bq_d, bk_d, bv_d, bo_d, out_d):
    with (
        tc.tile_pool(name="const", bufs=1) as constp,
        tc.tile_pool(name="wqk", bufs=1) as wqkp,
        tc.tile_pool(name="vpool", bufs=1) as vp,
        tc.tile_pool(name="otp", bufs=1) as otp,
        tc.tile_pool(name="qkt", bufs=2) as qktp,
        tc.tile_pool(name="ep", bufs=3) as ep,
        tc.tile_pool(name="normp", bufs=1) as normp,
        tc.tile_pool(name="xq", bufs=1) as xqp,
        tc.tile_pool(name="xk", bufs=1) as xkp,
        tc.tile_pool(name="ps_w", bufs=2, space="PSUM") as ps_w,
        tc.tile_pool(name="ps_sc", bufs=2, space="PSUM") as ps_sc,
        tc.tile_pool(name="ps_pv", bufs=2, space="PSUM") as ps_pv,
    ):
        # ----- constants
        identf = constp.tile([P, P], F32)
        make_identity(nc, identf[:])
        ident = constp.tile([P, P], BF16)
        nc.vector.tensor_copy(ident[:], identf[:])
        onesr_f = constp.tile([1, P], F32)
        nc.vector.memset(onesr_f[:], 1.0)
        ones_row = constp.tile([1, P], F32R)
        nc.vector.tensor_copy(ones_row[:], onesr_f[:])
        ones16 = constp.tile([P, H], F32)
        nc.vector.memset(ones16[:], 1.0)
        bq_t = constp.tile([P, MC], F32)
        nc.sync.dma_start(bq_t[:], bq_d.ap().rearrange("(c p) -> p c", p=P))
        bk_t = constp.tile([P, MC], F32)
        nc.sync.dma_start(bk_t[:], bk_d.ap().rearrange("(c p) -> p c", p=P))
        bv_t = constp.tile([P, MC], F32)
        nc.sync.dma_start(bv_t[:], bv_d.ap().rearrange("(c p) -> p c", p=P))

        # ----- persistent tiles
        V = [vp.tile([P, H * 65], BF16, tag=f"v{i}", name=f"v{i}")
             for i in range(KC)]
        OT = [otp.tile([P, SQ], BF16, tag=f"ot{i}", name=f"ot{i}")
              for i in range(DKC)]
        xqT = [xqp.tile([P, SQ], BF16, tag=f"xq{i}", name=f"xqt{i}")
               for i in range(MC)]
        xkT = [xkp.tile([P, S], BF16, tag=f"xk{i}", name=f"xkt{i}")
               for i in range(MC)]
        wq_t, wk_t = [], []
        wv_t, wo_t = [], []
        QT, KT = {}, {}
        cur_valT = {}
        vstate = {"ready": -1}

        # ---------- emission helpers ----------
        def emit_wcast(stg, w_d, mm, dst_pool, tag):
            raw = stg.tile([P, D], F32, tag="xin", bufs=2, name="wraw")
            nc.sync.dma_start(raw[:], w_d.ap()[mm * P:(mm + 1) * P, :])
            t = dst_pool.tile([P, D], BF16, tag=f"{tag}{mm}", name=f"{tag}{mm}")
            nc.vector.tensor_copy(t[:], raw[:])
            return t

        def emit_tgroup(stg, x_d, g, xT):
            """DMA 4 row-chunks of x_d, cast to bf16, PE-transpose into
            xT[mm][:, g*512:(g+1)*512]."""
            xbs = []
            for j in range(4):
                r = g * 4 + j
                xin = stg.tile([P, D], F32, tag="xin", bufs=2, name="xin")
                nc.sync.dma_start(xin[:], x_d.ap()[r * P:(r + 1) * P, :])
                xb = stg.tile([P, D], BF16, tag="xb", bufs=4, name="xb")
                nc.vector.tensor_copy(xb[:], xin[:])
                xbs.append(xb)
            for mm in range(MC):
                pst = ps_w.tile([P, 512], BF16, tag="w", name="pst")
                for j in range(4):
                    nc.tensor.transpose(
                        pst[:, j * P:(j + 1) * P],
                        xbs[j][:, mm * P:(mm + 1) * P], ident[:])
                nc.vector.tensor_copy(xT[mm][:, g * 512:(g + 1) * 512], pst[:])

        def emit_qproj_nh(p, nh):
            if p not in QT:
                QT[p] = qktp.tile([P, SQ], BF16, tag="qt", name=f"qt{p}")
            ps = ps_w.tile([P, 512], F32, tag="w", name="pq")
            for mm in range(MC):
                nc.tensor.matmul(
                    ps[:], wq_t[mm][:, p * P:(p + 1) * P],
                    xqT[mm][:, nh * 512:(nh + 1) * 512],
                    start=(mm == 0), stop=(mm == MC - 1))
            nc.vector.tensor_scalar_add(
                QT[p][:, nh * 512:(nh + 1) * 512], ps[:], bq_t[:, p:p + 1])

        def emit_kproj_nh(p, nh):
            if p not in KT:
                KT[p] = qktp.tile([P, S], BF16, tag="kt", name=f"kt{p}")
            ps = ps_w.tile([P, 512], F32, tag="w", name="pk")
            for mm in range(MC):
                nc.tensor.matmul(
                    ps[:], wk_t[mm][:, p * P:(p + 1) * P],
                    xkT[mm][:, nh * 512:(nh + 1) * 512],
                    start=(mm == 0), stop=(mm == MC - 1))
            nc.vector.tensor_scalar_add(
                KT[p][:, nh * 512:(nh + 1) * 512], ps[:], bk_t[:, p:p + 1])

        def mk_qk_items(p):
            items = []
            for nh in range(SQ // 512):
                items.append(lambda nh=nh: emit_qproj_nh(p, nh))
            for nh in range(S // 512):
                items.append(lambda nh=nh: emit_kproj_nh(p, nh))
            return items

        def mk_vgroup_item(stg, g):
            def f():
                xbs = []
                for j in range(4):
                    r = g * 4 + j
                    xin = stg.tile([P, D], F32, tag="xin", bufs=2, name="xin")
                    nc.sync.dma_start(xin[:], v_in.ap()[r * P:(r + 1) * P, :])
                    xb = stg.tile([P, D], BF16, tag="xb", bufs=4, name="xb")
                    nc.vector.tensor_copy(xb[:], xin[:])
                    xbs.append(xb)
                vts = [stg.tile([P, 512], BF16, tag=f"vt{mm}", bufs=1,
                                name=f"vt{mm}") for mm in range(MC)]
                for mm in range(MC):
                    pst = ps_w.tile([P, 512], BF16, tag="w", name="pst")
                    for j in range(4):
                        nc.tensor.transpose(
                            pst[:, j * P:(j + 1) * P],
                            xbs[j][:, mm * P:(mm + 1) * P], ident[:])
                    nc.vector.tensor_copy(vts[mm][:], pst[:])
                cur_valT[g] = vts
            return f

        def mk_vproj_item(sc, nh):
            def f():
                g, j = sc // 4, sc % 4
                vts = cur_valT[g]
                vx = V[sc].rearrange("p (h c) -> p h c", c=65)
                if nh == 0:
                    nc.vector.tensor_copy(
                        vx[:, :, 64:65],
                        ones16[:].rearrange("p (h c) -> p h c", c=1))
                ps = ps_w.tile([P, 512], F32, tag="w", name="pvps")
                for mm in range(MC):
                    nc.tensor.matmul(
                        ps[:], vts[mm][:, j * P:(j + 1) * P],
                        wv_t[mm][:, nh * 512:(nh + 1) * 512],
                        start=(mm == 0), stop=(mm == MC - 1))
                nc.vector.tensor_copy(
                    vx[:, 8 * nh:8 * nh + 8, 0:64],
                    ps[:].rearrange("p (h c) -> p h c", c=64))
                if nh == 1:
                    vstate["ready"] = sc
            return f

        def normalize(pair, qs, pv1, pv2):
            """Softmax-normalize both heads (row 64 of each psum = sums)."""
            for hh, pvp in ((0, pv1), (1, pv2)):
                rb = normp.tile([P, 512], F32, tag="rb", bufs=1, name="rb")
                nc.vector.tensor_copy(rb[64:65, :], pvp[64:65, :])
                r0 = normp.tile([1, 512], F32, tag="r0", bufs=1, name="r0")
                nc.gpsimd.tensor_copy(r0[:], rb[64:65, :])
                rr = normp.tile([1, 512], F32, tag="rr", bufs=1, name="rr")
                nc.vector.reciprocal_approx_fast(rr[:], r0[:])
                bc = normp.tile([64, 512], F32, tag="bc", bufs=2, name="bc")
                nc.gpsimd.partition_broadcast(bc[:], rr[:])
                if hh == 0:
                    osl = OT[pair][0:64, qs]
                    nc.vector.tensor_mul(osl, pvp[0:64, :], bc[:])
                    nc.vector.tensor_scalar_add(
                        osl, osl, bv_t[0:64, pair:pair + 1])
                else:
                    tmp = normp.tile([64, 512], BF16, tag="tmp", bufs=1,
                                     name="tmp")
                    nc.vector.tensor_mul(tmp[:], pvp[0:64, :], bc[:])
                    osl = OT[pair][64:128, qs]
                    nc.sync.dma_start(osl, tmp[:])
                    nc.vector.tensor_scalar_add(
                        osl, osl, bv_t[64:128, pair:pair + 1])

        vitems = deque()
        qkitems = deque()
        woitems = deque()
        slot_ctr = [0]

        def pump(need_block):
            """Emit deferred work: V items needed before the pending PV
            (emission-order correctness), then opportunistic fillers."""
            while vitems and vstate["ready"] < need_block:
                vitems.popleft()()
            slot_ctr[0] += 1
            if vitems:
                vitems.popleft()()
            elif qkitems and slot_ctr[0] % 3 == 0:
                qkitems.popleft()()
            elif woitems and slot_ctr[0] % 4 == 0:
                woitems.popleft()()

        def emit_pv(pair, k2, e1, e2, pv1, pv2):
            c1 = (2 * pair) * 65
            c2 = (2 * pair + 1) * 65
            first = k2 == 0
            last = k2 == KC // 2 - 1
            nc.tensor.matmul(pv1[0:65, :], V[2 * k2][:, c1:c1 + 65],
                             e1[:, 0:512], start=first, stop=False)
            nc.tensor.matmul(pv2[0:65, :], V[2 * k2][:, c2:c2 + 65],
                             e2[:, 0:512], start=first, stop=False)
            nc.tensor.matmul(pv1[0:65, :], V[2 * k2 + 1][:, c1:c1 + 65],
                             e1[:, 512:1024], start=False, stop=last)
            nc.tensor.matmul(pv2[0:65, :], V[2 * k2 + 1][:, c2:c2 + 65],
                             e2[:, 512:1024], start=False, stop=last)

        def attn_pair(p):
            for qt in range(SQ // 512):
                qs = slice(qt * 512, (qt + 1) * 512)
                pv1 = ps_pv.tile([P, 512], F32, tag="pv", name="pv1")
                pv2 = ps_pv.tile([P, 512], F32, tag="pv", name="pv2")
                pend = None
                for k2 in range(KC // 2):
                    ka = slice(2 * k2 * P, (2 * k2 + 1) * P)
                    kb = slice((2 * k2 + 1) * P, (2 * k2 + 2) * P)
                    s1 = ps_sc.tile([P, 1024], F32, tag="sc", name="s1")
                    s2 = ps_sc.tile([P, 1024], F32, tag="sc", name="s2")
                    nc.tensor.matmul(s1[:, 0:512], KT[p][0:64, ka],
                                     QT[p][0:64, qs], start=True, stop=True,
                                     tile_position=(0, 0))
                    nc.tensor.matmul(s2[:, 0:512], KT[p][64:128, ka],
                                     QT[p][64:128, qs], start=True, stop=True,
                                     tile_position=(64, 0))
                    nc.tensor.matmul(s1[:, 512:1024], KT[p][0:64, kb],
                                     QT[p][0:64, qs], start=True, stop=True,
                                     tile_position=(0, 0))
                    nc.tensor.matmul(s2[:, 512:1024], KT[p][64:128, kb],
                                     QT[p][64:128, qs], start=True, stop=True,
                                     tile_position=(64, 0))
                    e1 = ep.tile([P, 1024], BF16, tag="ea", name="e1")
                    e2 = ep.tile([P, 1024], BF16, tag="eb", name="e2")
                    nc.scalar.activation(e1[:], s1[:], EXP, scale=SCALE)
                    nc.scalar.activation(e2[:], s2[:], EXP, scale=SCALE)
                    pump(2 * (pend[0] if pend else 0) + 1)
                    if pend is not None:
                        emit_pv(p, pend[0], pend[1], pend[2], pv1, pv2)
                    pend = (k2, e1, e2)
                while vitems and vstate["ready"] < KC - 1:
                    vitems.popleft()()
                emit_pv(p, pend[0], pend[1], pend[2], pv1, pv2)
                normalize(p, qs, pv1, pv2)

        # ---------- emission ----------
        with tc.tile_pool(name="stg", bufs=1) as stg:
            # Q input pipeline + Wq + Q proj chunk 0
            for g in range(SQ // 512):
                emit_tgroup(stg, q_in, g, xqT)
            for mm in range(MC):
                wq_t.append(emit_wcast(stg, wq_d, mm, wqkp, "wq"))
            emit_qproj_nh(0, 0)
            emit_qproj_nh(0, 1)
            # K input pipeline + Wk + K proj chunk 0
            for g in range(S // 512):
                emit_tgroup(stg, k_in, g, xkT)
            for mm in range(MC):
                wk_t.append(emit_wcast(stg, wk_d, mm, wqkp, "wk"))
            for nh in range(S // 512):
                emit_kproj_nh(0, nh)

            # V pipeline as deferred items (woven into pair-0/1 attention)
            def mk_wv_item(mm):
                def f():
                    wv_t.append(emit_wcast(stg, wv_d, mm, stg, "wv"))
                return f
            for mm in range(MC):
                vitems.append(mk_wv_item(mm))
            for g in range(S // 512):
                vitems.append(mk_vgroup_item(stg, g))
                for j in range(4):
                    for nh in range(2):
                        vitems.append(mk_vproj_item(4 * g + j, nh))

            qkitems.extend(mk_qk_items(1))
            attn_pair(0)
            while vitems:
                vitems.popleft()()
            while qkitems:
                qkitems.popleft()()
            qkitems.extend(mk_qk_items(2))
            attn_pair(1)
            while qkitems:
                qkitems.popleft()()

        with tc.tile_pool(name="wop", bufs=1) as wop:
            # Wo load + cast, woven into pairs 2-3
            def mk_wo_item(mm):
                def f():
                    raw = wop.tile([P, D], F32, tag="woraw", bufs=2,
                                   name="woraw")
                    nc.sync.dma_start(raw[:], wo_d.ap()[mm * P:(mm + 1) * P, :])
                    t = wop.tile([P, D], BF16, tag=f"wo{mm}", name=f"wo{mm}")
                    nc.vector.tensor_copy(t[:], raw[:])
                    wo_t.append(t)
                return f
            for mm in range(MC):
                woitems.append(mk_wo_item(mm))

            def bo_item():
                bo_f = wop.tile([1, D], F32, tag="bof", name="bo_f")
                nc.sync.dma_start(bo_f[:], bo_d.ap().unsqueeze(0))
                bo_t = wop.tile([1, D], F32R, tag="bot", name="bo_t")
                nc.vector.tensor_copy(bo_t[:], bo_f[:])
                wo_t.append(bo_t)  # sentinel slot 8
            woitems.append(bo_item)

            for p in range(2, DKC):
                if p < DKC - 1:
                    qkitems.extend(mk_qk_items(p + 1))
                attn_pair(p)
                while qkitems:
                    qkitems.popleft()()
            while woitems:
                woitems.popleft()()
            bo_t = wo_t[8]

            # final projection: out = OT.T @ Wo + bo
            for nh in range(2):
                ns = slice(nh * 512, (nh + 1) * 512)
                for sc in range(SQ // P):
                    ss = slice(sc * P, (sc + 1) * P)
                    fps = ps_pv.tile([P, 512], F32, tag="pv", name="fps")
                    for dk in range(DKC):
                        nc.tensor.matmul(fps[:], OT[dk][:, ss],
                                         wo_t[dk][:, ns],
                                         start=(dk == 0), stop=False)
                    nc.tensor.matmul(fps[:], ones_row[:], bo_t[:, ns],
                                     start=False, stop=True)
                    ob = wop.tile([P, 512], F32, tag="ob", bufs=2, name="ob")
                    nc.vector.tensor_copy(ob[:], fps[:])
                    nc.sync.dma_start(out_d.ap()[ss, ns], ob[:])


def get_nc():
    global _CACHED_NC
    if _CACHED_NC is None:
        _CACHED_NC = build_nc()
    return _CACHED_NC


def run(inputs, **kwargs):
    """Run on 8 cores; returns (full_output, BassKernelResults)."""
    nc = get_nc()
    queries = np.ascontiguousarray(np.asarray(inputs["queries"], np.float32))
    keys = np.ascontiguousarray(np.asarray(inputs["keys"], np.float32))
    values = np.ascontiguousarray(np.asarray(inputs["values"], np.float32))
    base = {
        "wq": np.ascontiguousarray(np.asarray(inputs["Wq"], np.float32)),
        "wk": np.ascontiguousarray(np.asarray(inputs["Wk"], np.float32)),
        "wv": np.ascontiguousarray(np.asarray(inputs["Wv"], np.float32)),
        "wo": np.ascontiguousarray(np.asarray(inputs["Wo"], np.float32)),
        "bq": np.ascontiguousarray(np.asarray(inputs["bq"], np.float32)),
        "bk": np.ascontiguousarray(np.asarray(inputs["bk"], np.float32)),
        "bv": np.ascontiguousarray(np.asarray(inputs["bv"], np.float32)),
        "bo": np.ascontiguousarray(np.asarray(inputs["bo"], np.float32)),
    }
    in_maps = []
    for c in range(N_CORES):
        b, qh = c // 2, c % 2
        m = dict(base)
        m["q_in"] = np.ascontiguousarray(queries[b, qh * SQ:(qh + 1) * SQ])
        m["k_in"] = keys[b]
        m["v_in"] = values[b]
        in_maps.append(m)
    res = bass_utils.run_bass_kernel_spmd(
        nc, in_maps, core_ids=list(range(N_CORES)), **kwargs)
    out = np.empty((B, S, D), np.float32)
    for c in range(N_CORES):
        b, qh = c // 2, c % 2
        out[b, qh * SQ:(qh + 1) * SQ] = res.results[c]["out"]
    return out, res


def kernel(**inputs):
    out, _ = run(inputs)
    return out


if __name__ == "__main__":
    rng = np.random.default_rng(0)
    ins = {
        "queries": rng.standard_normal((B, S, D), dtype=np.float32),
        "keys": rng.standard_normal((B, S, D), dtype=np.float32),
        "values": rng.standard_normal((B, S, D), dtype=np.float32),
        "Wq": (rng.standard_normal((D, D), dtype=np.float32) / 32),
        "bq": np.zeros(D, np.float32),
        "Wk": (rng.standard_normal((D, D), dtype=np.float32) / 32),
        "bk": np.zeros(D, np.float32),
        "Wv": (rng.standard_normal((D, D), dtype=np.float32) / 32),
        "bv": np.zeros(D, np.float32),
        "Wo": (rng.standard_normal((D, D), dtype=np.float32) / 32),
        "bo": np.zeros(D, np.float32),
    }
    out = kernel(**ins)
    print("out", out.shape, out.dtype, np.abs(out).mean())


# revision 12
# speedup vs baseline: 1.2670x; 1.1585x over previous
"""Multi-head attention (B=4, S=2048, D=1024, H=16) on 8 TRN2 NeuronCores.

Sharding: core c handles batch b = c//2 and query-half qh = c%2 (1024 query
rows), with K/V projection for its batch replicated across the 2 cores that
share the batch. Zero inter-core communication; host just slices inputs and
concatenates outputs.

Per-core dataflow (all matmuls bf16, accumulated in f32 PSUM):
  1. Inputs are DVE-cast to bf16, PE-transposed to X^T layout.
  2. Projections: Q^T/K^T = W^T chunks @ X^T (bias fused in the DVE
     eviction); V = X^T-chunks(stationary) @ Wv with a ones column per head
     (softmax denominators ride the PV matmul).
  3. Attention per head-pair, per q-tile(512): scores^T via row-packed
     bf16 matmuls (2 heads in PE quadrants), exp on ScalarE (the ONLY
     ScalarE work - it is the critical resource) into bf16, PV, then a fast
     PSUM->SBUF evict and DVE/GPSIMD reciprocal-broadcast normalization.
  4. Final: out = O^T-chunks.T @ Wo (+bo via K=1 ones matmul).

Scheduling: everything downstream of the input DMA is software-pipelined.
The K/V input pipelines, projection chunks for head-pair p+1, the Wo load
and the first half of the final projection are emitted as need-driven
"filler" items interleaved into the attention slots of earlier pairs, so
the PE and ScalarE streams overlap end-to-end instead of running in serial
phases (keeps the PE HAM clock-gate warm).
"""

from collections import deque

import numpy as np

import concourse.bacc as bacc
import concourse.mybir as mybir
import concourse.tile as tile
from concourse import bass_utils
from concourse.masks import make_identity

F32 = mybir.dt.float32
F32R = mybir.dt.float32r
BF16 = mybir.dt.bfloat16
EXP = mybir.ActivationFunctionType.Exp

B, S, D, H = 4, 2048, 1024, 16
SQ = 1024          # query rows per core
P = 128
MC = D // P        # 8 m-chunks (contraction of projections)
DKC = D // P       # 8 dk-chunks == head pairs
KC = S // P        # 16 key chunks
SCALE = 1.0 / 32.0  # 1/sqrt(D_K)
N_CORES = 8

_CACHED_NC = None


def build_nc():
    nc = bacc.Bacc("TRN2", target_bir_lowering=False, debug=False,
                   num_devices=N_CORES)
    q_in = nc.dram_tensor("q_in", [SQ, D], F32, kind="ExternalInput")
    k_in = nc.dram_tensor("k_in", [S, D], F32, kind="ExternalInput")
    v_in = nc.dram_tensor("v_in", [S, D], F32, kind="ExternalInput")
    wq_d = nc.dram_tensor("wq", [D, D], F32, kind="ExternalInput")
    wk_d = nc.dram_tensor("wk", [D, D], F32, kind="ExternalInput")
    wv_d = nc.dram_tensor("wv", [D, D], F32, kind="ExternalInput")
    wo_d = nc.dram_tensor("wo", [D, D], F32, kind="ExternalInput")
    bq_d = nc.dram_tensor("bq", [D], F32, kind="ExternalInput")
    bk_d = nc.dram_tensor("bk", [D], F32, kind="ExternalInput")
    bv_d = nc.dram_tensor("bv", [D], F32, kind="ExternalInput")
    bo_d = nc.dram_tensor("bo", [D], F32, kind="ExternalInput")
    out_d = nc.dram_tensor("out", [SQ, D], F32, kind="ExternalOutput")

    with tile.TileContext(nc) as tc:
        _build_body(nc, tc, q_in, k_in, v_in, wq_d, wk_d, wv_d, wo_d,
                    bq_d, bk_d, bv_d, bo_d, out_d)
    nc.compile()
    return nc


def _build_body(nc, tc, q_in, k_in, v_in, wq_d, wk_d, wv_d, wo_d,
                bq_d, bk_d, bv_d, bo_d, out_d):
    with (
        tc.tile_pool(name="const", bufs=1) as constp,
        tc.tile_pool(name="wqk", bufs=1) as wqkp,
        tc.tile_pool(name="vpool", bufs=1) as vp,
        tc.tile_pool(name="otp", bufs=1) as otp,
        tc.tile_pool(name="qkt", bufs=2) as qktp,
        tc.tile_pool(name="ep", bufs=3) as ep,
        tc.tile_pool(name="normp", bufs=1) as normp,
        tc.tile_pool(name="xq", bufs=1) as xqp,
        tc.tile_pool(name="xk", bufs=1) as xkp,
        tc.tile_pool(name="ps_w", bufs=2, space="PSUM") as ps_w,
        tc.tile_pool(name="ps_sc", bufs=2, space="PSUM") as ps_sc,
        tc.tile_pool(name="ps_pv", bufs=2, space="PSUM") as ps_pv,
    ):
        # ----- constants
        ident = constp.tile([P, P], BF16)
        ones_row_cell = [None]
        ones16 = constp.tile([P, H], F32)
        nc.vector.memset(ones16[:], 1.0)
        bq_t = constp.tile([P, MC], F32)
        nc.sync.dma_start(bq_t[:], bq_d.ap().rearrange("(c p) -> p c", p=P))
        bk_t = constp.tile([P, MC], F32)
        nc.sync.dma_start(bk_t[:], bk_d.ap().rearrange("(c p) -> p c", p=P))
        bv_t = constp.tile([P, MC], F32)
        nc.sync.dma_start(bv_t[:], bv_d.ap().rearrange("(c p) -> p c", p=P))

        # ----- persistent tiles
        V = [vp.tile([P, H * 65], BF16, tag=f"v{i}", name=f"v{i}")
             for i in range(KC)]
        OT = [otp.tile([P, SQ], BF16, tag=f"ot{i}", name=f"ot{i}")
              for i in range(DKC)]
        xqT = [xqp.tile([P, SQ], BF16, tag=f"xq{i}", name=f"xqt{i}")
               for i in range(MC)]
        xkT = [xkp.tile([P, S], BF16, tag=f"xk{i}", name=f"xkt{i}")
               for i in range(MC)]
        wq_t, wk_t = [], []
        wv_t, wo_t = [], []
        QT, KT = {}, {}
        cur_valT = {}
        vstate = {"ready": -1}
        kstate = {"ready": 0}   # nh chunks of KT[0] emitted

        # ---------- emission helpers ----------
        def emit_wcast(stg, w_d, mm, dst_pool, tag):
            raw = stg.tile([P, D], F32, tag="xin", bufs=2, name="wraw")
            nc.sync.dma_start(raw[:], w_d.ap()[mm * P:(mm + 1) * P, :])
            t = dst_pool.tile([P, D], BF16, tag=f"{tag}{mm}", name=f"{tag}{mm}")
            nc.vector.tensor_copy(t[:], raw[:])
            return t

        def emit_tgroup(stg, x_d, g, xT):
            """DMA 4 row-chunks of x_d, cast to bf16, PE-transpose into
            xT[mm][:, g*512:(g+1)*512]."""
            xbs = []
            for j in range(4):
                r = g * 4 + j
                xin = stg.tile([P, D], F32, tag="xin", bufs=2, name="xin")
                nc.sync.dma_start(xin[:], x_d.ap()[r * P:(r + 1) * P, :])
                xb = stg.tile([P, D], BF16, tag="xb", bufs=4, name="xb")
                nc.vector.tensor_copy(xb[:], xin[:])
                xbs.append(xb)
            for mm in range(MC):
                pst = ps_w.tile([P, 512], BF16, tag="w", name="pst")
                for j in range(4):
                    nc.tensor.transpose(
                        pst[:, j * P:(j + 1) * P],
                        xbs[j][:, mm * P:(mm + 1) * P], ident[:])
                nc.vector.tensor_copy(xT[mm][:, g * 512:(g + 1) * 512], pst[:])

        def emit_qproj_nh(p, nh):
            if p not in QT:
                QT[p] = qktp.tile([P, SQ], BF16, tag="qt", name=f"qt{p}")
            ps = ps_w.tile([P, 512], F32, tag="w", name="pq")
            for mm in range(MC):
                nc.tensor.matmul(
                    ps[:], wq_t[mm][:, p * P:(p + 1) * P],
                    xqT[mm][:, nh * 512:(nh + 1) * 512],
                    start=(mm == 0), stop=(mm == MC - 1))
            nc.vector.tensor_scalar_add(
                QT[p][:, nh * 512:(nh + 1) * 512], ps[:], bq_t[:, p:p + 1])

        def emit_kproj_nh(p, nh):
            if p not in KT:
                KT[p] = qktp.tile([P, S], BF16, tag="kt", name=f"kt{p}")
            ps = ps_w.tile([P, 512], F32, tag="w", name="pk")
            for mm in range(MC):
                nc.tensor.matmul(
                    ps[:], wk_t[mm][:, p * P:(p + 1) * P],
                    xkT[mm][:, nh * 512:(nh + 1) * 512],
                    start=(mm == 0), stop=(mm == MC - 1))
            nc.vector.tensor_scalar_add(
                KT[p][:, nh * 512:(nh + 1) * 512], ps[:], bk_t[:, p:p + 1])

        def mk_qk_items(p):
            items = []
            for nh in range(SQ // 512):
                items.append(lambda nh=nh: emit_qproj_nh(p, nh))
            for nh in range(S // 512):
                items.append(lambda nh=nh: emit_kproj_nh(p, nh))
            return items

        def mk_vgroup_item(stg, g):
            def f():
                xbs = []
                for j in range(4):
                    r = g * 4 + j
                    xin = stg.tile([P, D], F32, tag="xin", bufs=2, name="xin")
                    nc.sync.dma_start(xin[:], v_in.ap()[r * P:(r + 1) * P, :])
                    xb = stg.tile([P, D], BF16, tag="xb", bufs=4, name="xb")
                    nc.vector.tensor_copy(xb[:], xin[:])
                    xbs.append(xb)
                vts = [stg.tile([P, 512], BF16, tag=f"vt{mm}", bufs=1,
                                name=f"vt{mm}") for mm in range(MC)]
                for mm in range(MC):
                    pst = ps_w.tile([P, 512], BF16, tag="w", name="pst")
                    for j in range(4):
                        nc.tensor.transpose(
                            pst[:, j * P:(j + 1) * P],
                            xbs[j][:, mm * P:(mm + 1) * P], ident[:])
                    nc.vector.tensor_copy(vts[mm][:], pst[:])
                cur_valT[g] = vts
            return f

        def mk_vproj_item(sc, nh):
            def f():
                g, j = sc // 4, sc % 4
                vts = cur_valT[g]
                vx = V[sc].rearrange("p (h c) -> p h c", c=65)
                if nh == 0:
                    nc.vector.tensor_copy(
                        vx[:, :, 64:65],
                        ones16[:].rearrange("p (h c) -> p h c", c=1))
                ps = ps_w.tile([P, 512], F32, tag="w", name="pvps")
                for mm in range(MC):
                    nc.tensor.matmul(
                        ps[:], vts[mm][:, j * P:(j + 1) * P],
                        wv_t[mm][:, nh * 512:(nh + 1) * 512],
                        start=(mm == 0), stop=(mm == MC - 1))
                nc.vector.tensor_copy(
                    vx[:, 8 * nh:8 * nh + 8, 0:64],
                    ps[:].rearrange("p (h c) -> p h c", c=64))
                if nh == 1:
                    vstate["ready"] = sc
            return f

        def normalize(pair, qs, pv1, pv2):
            """Evict PV psums wholesale (frees the banks fast), then
            softmax-normalize both heads (row 64 = sums)."""
            for hh, pvp in ((0, pv1), (1, pv2)):
                pb = normp.tile([65, 512], F32, tag="pb", bufs=2, name="pb")
                nc.vector.tensor_copy(pb[:], pvp[0:65, :])
                r0 = normp.tile([1, 512], F32, tag="r0", bufs=1, name="r0")
                nc.gpsimd.tensor_copy(r0[:], pb[64:65, :])
                rr = normp.tile([1, 512], F32, tag="rr", bufs=1, name="rr")
                nc.vector.reciprocal_approx_fast(rr[:], r0[:])
                bc = normp.tile([64, 512], F32, tag="bc", bufs=2, name="bc")
                nc.gpsimd.partition_broadcast(bc[:], rr[:])
                if hh == 0:
                    osl = OT[pair][0:64, qs]
                    nc.vector.tensor_mul(osl, pb[0:64, :], bc[:])
                    nc.vector.tensor_scalar_add(
                        osl, osl, bv_t[0:64, pair:pair + 1])
                else:
                    tmp = normp.tile([64, 512], BF16, tag="tmp", bufs=2,
                                     name="tmp")
                    nc.vector.tensor_mul(tmp[:], pb[0:64, :], bc[:])
                    osl = OT[pair][64:128, qs]
                    nc.sync.dma_start(osl, tmp[:])
                    nc.vector.tensor_scalar_add(
                        osl, osl, bv_t[64:128, pair:pair + 1])

        def emit_final_item(nh, sc):
            ns = slice(nh * 512, (nh + 1) * 512)
            ss = slice(sc * P, (sc + 1) * P)
            fps = ps_w.tile([P, 512], F32, tag="w", name="fps")
            for dk in range(DKC):
                nc.tensor.matmul(fps[:], OT[dk][:, ss], wo_t[dk][:, ns],
                                 start=(dk == 0), stop=False)
            nc.tensor.matmul(fps[:], ones_row_cell[0][:], wo_t[8][:, ns],
                             start=False, stop=True)
            ob = final_pool[0].tile([P, 512], F32, tag="ob", bufs=2, name="ob")
            nc.vector.tensor_copy(ob[:], fps[:])
            nc.sync.dma_start(out_d.ap()[ss, ns], ob[:])

        final_pool = [None]
        vitems = deque()
        kqstart = deque()
        qkitems = deque()
        woitems = deque()
        lateitems = deque()
        slot_ctr = [0]

        def pump_opportunistic():
            slot_ctr[0] += 1
            if kqstart:
                kqstart.popleft()()
            elif vitems:
                vitems.popleft()()
            elif qkitems and slot_ctr[0] % 3 == 0:
                qkitems.popleft()()
            elif woitems and slot_ctr[0] % 4 == 0:
                woitems.popleft()()
            elif lateitems:
                lateitems.popleft()()

        def emit_pv(pair, k2, e1, e2, pv1, pv2):
            c1 = (2 * pair) * 65
            c2 = (2 * pair + 1) * 65
            first = k2 == 0
            last = k2 == KC // 2 - 1
            nc.tensor.matmul(pv1[0:65, :], V[2 * k2][:, c1:c1 + 65],
                             e1[:, 0:512], start=first, stop=False)
            nc.tensor.matmul(pv2[0:65, :], V[2 * k2][:, c2:c2 + 65],
                             e2[:, 0:512], start=first, stop=False)
            nc.tensor.matmul(pv1[0:65, :], V[2 * k2 + 1][:, c1:c1 + 65],
                             e1[:, 512:1024], start=False, stop=last)
            nc.tensor.matmul(pv2[0:65, :], V[2 * k2 + 1][:, c2:c2 + 65],
                             e2[:, 512:1024], start=False, stop=last)

        def attn_half(p, qt):
                if p == 0 and qt == 1:
                    while kqstart:
                        kqstart.popleft()()
                qs = slice(qt * 512, (qt + 1) * 512)
                pv1 = ps_pv.tile([P, 512], F32, tag="pv", name="pv1")
                pv2 = ps_pv.tile([P, 512], F32, tag="pv", name="pv2")
                pend = None
                for k2 in range(KC // 2):
                    if p == 0:
                        while kqstart and kstate["ready"] < k2 // 2 + 1:
                            kqstart.popleft()()
                    ka = slice(2 * k2 * P, (2 * k2 + 1) * P)
                    kb = slice((2 * k2 + 1) * P, (2 * k2 + 2) * P)
                    s1 = ps_sc.tile([P, 1024], F32, tag="sc", name="s1")
                    s2 = ps_sc.tile([P, 1024], F32, tag="sc", name="s2")
                    nc.tensor.matmul(s1[:, 0:512], KT[p][0:64, ka],
                                     QT[p][0:64, qs], start=True, stop=True,
                                     tile_position=(0, 0))
                    nc.tensor.matmul(s2[:, 0:512], KT[p][64:128, ka],
                                     QT[p][64:128, qs], start=True, stop=True,
                                     tile_position=(64, 0))
                    nc.tensor.matmul(s1[:, 512:1024], KT[p][0:64, kb],
                                     QT[p][0:64, qs], start=True, stop=True,
                                     tile_position=(0, 0))
                    nc.tensor.matmul(s2[:, 512:1024], KT[p][64:128, kb],
                                     QT[p][64:128, qs], start=True, stop=True,
                                     tile_position=(64, 0))
                    e1 = ep.tile([P, 1024], BF16, tag="ea", name="e1")
                    e2 = ep.tile([P, 1024], BF16, tag="eb", name="e2")
                    nc.scalar.activation(e1[:], s1[:], EXP, scale=SCALE)
                    nc.scalar.activation(e2[:], s2[:], EXP, scale=SCALE)
                    if pend is not None:
                        while vitems and vstate["ready"] < 2 * pend[0] + 1:
                            vitems.popleft()()
                    pump_opportunistic()
                    if pend is not None:
                        emit_pv(p, pend[0], pend[1], pend[2], pv1, pv2)
                    pend = (k2, e1, e2)
                while vitems and vstate["ready"] < KC - 1:
                    vitems.popleft()()
                emit_pv(p, pend[0], pend[1], pend[2], pv1, pv2)
                normalize(p, qs, pv1, pv2)

        def attn_pair(p):
            attn_half(p, 0)
            attn_half(p, 1)

        # ---------- emission ----------
        with tc.tile_pool(name="stg", bufs=1) as stg:
            identf = stg.tile([P, P], F32, tag="identf", name="identf")
            make_identity(nc, identf[:])
            nc.vector.tensor_copy(ident[:], identf[:])
            # pipelined startup: Q group 0 + Wq + Qproj(0,nh0); K group 0 +
            # Wk + Kproj(0,nh0); the rest of the Q/K pipeline rides the
            # pair-0 attention slots as need-driven items.
            emit_tgroup(stg, q_in, 0, xqT)
            for mm in range(MC):
                wq_t.append(emit_wcast(stg, wq_d, mm, wqkp, "wq"))
            emit_qproj_nh(0, 0)
            emit_tgroup(stg, k_in, 0, xkT)
            for mm in range(MC):
                wk_t.append(emit_wcast(stg, wk_d, mm, wqkp, "wk"))
            emit_kproj_nh(0, 0)
            kstate["ready"] = 1

            def mk_kstart_item(g):
                def f():
                    emit_tgroup(stg, k_in, g, xkT)
                    emit_kproj_nh(0, g)
                    kstate["ready"] = g + 1
                return f

            def qg1_item():
                emit_tgroup(stg, q_in, 1, xqT)
                emit_qproj_nh(0, 1)

            kqstart.append(mk_kstart_item(1))
            kqstart.append(qg1_item)
            kqstart.append(mk_kstart_item(2))
            kqstart.append(mk_kstart_item(3))

            def mk_wv_item(mm):
                def f():
                    wv_t.append(emit_wcast(stg, wv_d, mm, stg, "wv"))
                return f
            for mm in range(MC):
                vitems.append(mk_wv_item(mm))
            for g in range(S // 512):
                vitems.append(mk_vgroup_item(stg, g))
                for j in range(4):
                    for nh in range(2):
                        vitems.append(mk_vproj_item(4 * g + j, nh))

            qkitems.extend(mk_qk_items(1))
            attn_pair(0)
            while kqstart:
                kqstart.popleft()()
            while vitems:
                vitems.popleft()()
            while qkitems:
                qkitems.popleft()()
            qkitems.extend(mk_qk_items(2))
            attn_pair(1)
            while qkitems:
                qkitems.popleft()()

        with tc.tile_pool(name="wop", bufs=1) as wop:
            final_pool[0] = wop

            def mk_wo_item(mm):
                def f():
                    raw = wop.tile([P, D], F32, tag="woraw", bufs=2,
                                   name="woraw")
                    nc.sync.dma_start(raw[:], wo_d.ap()[mm * P:(mm + 1) * P, :])
                    t = wop.tile([P, D], BF16, tag=f"wo{mm}", name=f"wo{mm}")
                    nc.vector.tensor_copy(t[:], raw[:])
                    wo_t.append(t)
                return f
            for mm in range(MC):
                woitems.append(mk_wo_item(mm))

            def bo_item():
                bo_f = wop.tile([1, D], F32, tag="bof", name="bo_f")
                nc.sync.dma_start(bo_f[:], bo_d.ap().unsqueeze(0))
                bo_t = wop.tile([1, D], F32R, tag="bot", name="bo_t")
                nc.vector.tensor_copy(bo_t[:], bo_f[:])
                wo_t.append(bo_t)  # sentinel slot 8
                onesr_f = wop.tile([1, P], F32, tag="onesf", name="onesr_f")
                nc.vector.memset(onesr_f[:], 1.0)
                ones_row = wop.tile([1, P], F32R, tag="ones", name="ones_row")
                nc.vector.tensor_copy(ones_row[:], onesr_f[:])
                ones_row_cell[0] = ones_row
            woitems.append(bo_item)

            for p in range(2, DKC - 1):
                qkitems.extend(mk_qk_items(p + 1))
                attn_pair(p)
                while qkitems:
                    qkitems.popleft()()
            while woitems:
                woitems.popleft()()
            attn_half(DKC - 1, 0)
            # first-half (qt0) final projection rides pair-7 qt1
            for nh in range(2):
                for sc in range(4):
                    lateitems.append(
                        lambda nh=nh, sc=sc: emit_final_item(nh, sc))
            attn_half(DKC - 1, 1)
            while woitems:
                woitems.popleft()()
            while lateitems:
                lateitems.popleft()()
            for nh in range(2):
                for sc in range(4, SQ // P):
                    emit_final_item(nh, sc)


def get_nc():
    global _CACHED_NC
    if _CACHED_NC is None:
        _CACHED_NC = build_nc()
    return _CACHED_NC


def run(inputs, **kwargs):
    """Run on 8 cores; returns (full_output, BassKernelResults)."""
    nc = get_nc()
    queries = np.ascontiguousarray(np.asarray(inputs["queries"], np.float32))
    keys = np.ascontiguousarray(np.asarray(inputs["keys"], np.float32))
    values = np.ascontiguousarray(np.asarray(inputs["values"], np.float32))
    base = {
        "wq": np.ascontiguousarray(np.asarray(inputs["Wq"], np.float32)),
        "wk": np.ascontiguousarray(np.asarray(inputs["Wk"], np.float32)),
        "wv": np.ascontiguousarray(np.asarray(inputs["Wv"], np.float32)),
        "wo": np.ascontiguousarray(np.asarray(inputs["Wo"], np.float32)),
        "bq": np.ascontiguousarray(np.asarray(inputs["bq"], np.float32)),
        "bk": np.ascontiguousarray(np.asarray(inputs["bk"], np.float32)),
        "bv": np.ascontiguousarray(np.asarray(inputs["bv"], np.float32)),
        "bo": np.ascontiguousarray(np.asarray(inputs["bo"], np.float32)),
    }
    in_maps = []
    for c in range(N_CORES):
        b, qh = c // 2, c % 2
        m = dict(base)
        m["q_in"] = np.ascontiguousarray(queries[b, qh * SQ:(qh + 1) * SQ])
        m["k_in"] = keys[b]
        m["v_in"] = values[b]
        in_maps.append(m)
    res = bass_utils.run_bass_kernel_spmd(
        nc, in_maps, core_ids=list(range(N_CORES)), **kwargs)
    out = np.empty((B, S, D), np.float32)
    for c in range(N_CORES):
        b, qh = c // 2, c % 2
        out[b, qh * SQ:(qh + 1) * SQ] = res.results[c]["out"]
    return out, res


def kernel(**inputs):
    out, _ = run(inputs)
    return out


if __name__ == "__main__":
    rng = np.random.default_rng(0)
    ins = {
        "queries": rng.standard_normal((B, S, D), dtype=np.float32),
        "keys": rng.standard_normal((B, S, D), dtype=np.float32),
        "values": rng.standard_normal((B, S, D), dtype=np.float32),
        "Wq": (rng.standard_normal((D, D), dtype=np.float32) / 32),
        "bq": np.zeros(D, np.float32),
        "Wk": (rng.standard_normal((D, D), dtype=np.float32) / 32),
        "bk": np.zeros(D, np.float32),
        "Wv": (rng.standard_normal((D, D), dtype=np.float32) / 32),
        "bv": np.zeros(D, np.float32),
        "Wo": (rng.standard_normal((D, D), dtype=np.float32) / 32),
        "bo": np.zeros(D, np.float32),
    }
    out = kernel(**ins)
    print("out", out.shape, out.dtype, np.abs(out).mean())


# revision 18
# speedup vs baseline: 1.2912x; 1.0191x over previous
"""Multi-head attention (B=4, S=2048, D=1024, H=16) on 8 TRN2 NeuronCores.

Sharding: core c handles batch b = c//2 and query-half qh = c%2 (1024 query
rows), with K/V projection for its batch replicated across the 2 cores that
share the batch. Zero inter-core communication; host just slices inputs and
concatenates outputs.

Per-core dataflow (all matmuls bf16, accumulated in f32 PSUM):
  1. Inputs are DVE-cast to bf16, PE-transposed to X^T layout.
  2. Projections: Q^T/K^T = W^T chunks @ X^T (bias fused in the DVE
     eviction); V = X^T-chunks(stationary) @ Wv with a ones column per head
     (softmax denominators ride the PV matmul).
  3. Attention per head-pair, per q-tile(512): scores^T via row-packed
     bf16 matmuls (2 heads in PE quadrants), exp on ScalarE (the ONLY
     ScalarE work - it is the critical resource) into bf16, PV, then a fast
     PSUM->SBUF evict and DVE/GPSIMD reciprocal-broadcast normalization.
  4. Final: out = O^T-chunks.T @ Wo (+bo via K=1 ones matmul).

Scheduling: everything downstream of the input DMA is software-pipelined.
The K/V input pipelines, projection chunks for head-pair p+1, the Wo load
and the first half of the final projection are emitted as need-driven
"filler" items interleaved into the attention slots of earlier pairs, so
the PE and ScalarE streams overlap end-to-end instead of running in serial
phases (keeps the PE HAM clock-gate warm).
"""

from collections import deque

import numpy as np

import concourse.bacc as bacc
import concourse.mybir as mybir
import concourse.tile as tile
from concourse import bass_utils
from concourse.masks import make_identity

F32 = mybir.dt.float32
F32R = mybir.dt.float32r
BF16 = mybir.dt.bfloat16
EXP = mybir.ActivationFunctionType.Exp

B, S, D, H = 4, 2048, 1024, 16
SQ = 1024          # query rows per core
P = 128
MC = D // P        # 8 m-chunks (contraction of projections)
DKC = D // P       # 8 dk-chunks == head pairs
KC = S // P        # 16 key chunks
SCALE = 1.0 / 32.0  # 1/sqrt(D_K)
N_CORES = 8

_CACHED_NC = None


def build_nc():
    nc = bacc.Bacc("TRN2", target_bir_lowering=False, debug=False,
                   num_devices=N_CORES)
    q_in = nc.dram_tensor("q_in", [SQ, D], F32, kind="ExternalInput")
    k_in = nc.dram_tensor("k_in", [S, D], F32, kind="ExternalInput")
    v_in = nc.dram_tensor("v_in", [S, D], F32, kind="ExternalInput")
    wq_d = nc.dram_tensor("wq", [D, D], F32, kind="ExternalInput")
    wk_d = nc.dram_tensor("wk", [D, D], F32, kind="ExternalInput")
    wv_d = nc.dram_tensor("wv", [D, D], F32, kind="ExternalInput")
    wo_d = nc.dram_tensor("wo", [D, D], F32, kind="ExternalInput")
    bq_d = nc.dram_tensor("bq", [D], F32, kind="ExternalInput")
    bk_d = nc.dram_tensor("bk", [D], F32, kind="ExternalInput")
    bv_d = nc.dram_tensor("bv", [D], F32, kind="ExternalInput")
    bo_d = nc.dram_tensor("bo", [D], F32, kind="ExternalInput")
    out_d = nc.dram_tensor("out", [SQ, D], F32, kind="ExternalOutput")

    with tile.TileContext(nc) as tc:
        _build_body(nc, tc, q_in, k_in, v_in, wq_d, wk_d, wv_d, wo_d,
                    bq_d, bk_d, bv_d, bo_d, out_d)
    nc.compile()
    return nc


def _build_body(nc, tc, q_in, k_in, v_in, wq_d, wk_d, wv_d, wo_d,
                bq_d, bk_d, bv_d, bo_d, out_d):
    with (
        tc.tile_pool(name="const", bufs=1) as constp,
        tc.tile_pool(name="wqk", bufs=1) as wqkp,
        tc.tile_pool(name="vpool", bufs=1) as vp,
        tc.tile_pool(name="otp", bufs=1) as otp,
        tc.tile_pool(name="qkt", bufs=2) as qktp,
        tc.tile_pool(name="ep", bufs=3) as ep,
        tc.tile_pool(name="normp", bufs=1) as normp,
        tc.tile_pool(name="xq", bufs=1) as xqp,
        tc.tile_pool(name="xk", bufs=1) as xkp,
        tc.tile_pool(name="ps_w", bufs=2, space="PSUM") as ps_w,
        tc.tile_pool(name="ps_sc", bufs=2, space="PSUM") as ps_sc,
        tc.tile_pool(name="ps_pv", bufs=2, space="PSUM") as ps_pv,
    ):
        # ----- constants
        ones_row_cell = [None]
        ident_cell = [None]
        ones16 = constp.tile([P, H], F32)
        nc.vector.memset(ones16[:], 1.0)
        bq_t = constp.tile([P, MC], F32)
        nc.sync.dma_start(bq_t[:], bq_d.ap().rearrange("(c p) -> p c", p=P))
        bk_t = constp.tile([P, MC], F32)
        nc.sync.dma_start(bk_t[:], bk_d.ap().rearrange("(c p) -> p c", p=P))
        bv_t = constp.tile([P, MC], F32)
        nc.sync.dma_start(bv_t[:], bv_d.ap().rearrange("(c p) -> p c", p=P))

        # ----- persistent tiles
        V = [vp.tile([P, H * 65], BF16, tag=f"v{i}", name=f"v{i}")
             for i in range(KC)]
        OT = [otp.tile([P, SQ], BF16, tag=f"ot{i}", name=f"ot{i}")
              for i in range(DKC)]
        xqT = [xqp.tile([P, SQ], BF16, tag=f"xq{i}", name=f"xqt{i}")
               for i in range(MC)]
        xkT = [xkp.tile([P, S], BF16, tag=f"xk{i}", name=f"xkt{i}")
               for i in range(MC)]
        wq_t, wk_t = [], []
        wv_t, wo_t = [], []
        QT, KT = {}, {}
        cur_valT = {}
        vstate = {"ready": -1}
        kstate = {"ready": 0}   # nh chunks of KT[0] emitted

        # ---------- emission helpers ----------
        def emit_wcast(stg, w_d, mm, dst_pool, tag):
            raw = stg.tile([P, D], F32, tag="xin", bufs=4, name="wraw")
            nc.sync.dma_start(raw[:], w_d.ap()[mm * P:(mm + 1) * P, :])
            t = dst_pool.tile([P, D], BF16, tag=f"{tag}{mm}", name=f"{tag}{mm}")
            nc.vector.tensor_copy(t[:], raw[:])
            return t

        def emit_tgroup(stg, x_d, g, xT):
            """DMA 4 row-chunks of x_d, PE-transpose (f32r view) into
            bf16 xT[mm][:, g*512:(g+1)*512] (cast happens on eviction)."""
            xbs = []
            for j in range(4):
                r = g * 4 + j
                xin = stg.tile([P, D], F32, tag="xin", bufs=4, name="xin")
                nc.sync.dma_start(xin[:], x_d.ap()[r * P:(r + 1) * P, :])
                xbs.append(xin)
            for mm in range(MC):
                pst = ps_w.tile([P, 512], F32, tag="w", name="pst")
                for j in range(4):
                    nc.tensor.transpose(
                        pst[:, j * P:(j + 1) * P],
                        xbs[j][:, mm * P:(mm + 1) * P], ident_cell[0])
                nc.vector.tensor_copy(xT[mm][:, g * 512:(g + 1) * 512], pst[:])

        def emit_qproj_nh(p, nh):
            if p not in QT:
                QT[p] = qktp.tile([P, SQ], BF16, tag="qt", name=f"qt{p}")
            ps = ps_w.tile([P, 512], F32, tag="w", name="pq")
            for mm in range(MC):
                nc.tensor.matmul(
                    ps[:], wq_t[mm][:, p * P:(p + 1) * P],
                    xqT[mm][:, nh * 512:(nh + 1) * 512],
                    start=(mm == 0), stop=(mm == MC - 1))
            nc.vector.tensor_scalar_add(
                QT[p][:, nh * 512:(nh + 1) * 512], ps[:], bq_t[:, p:p + 1])

        def emit_kproj_nh(p, nh):
            if p not in KT:
                KT[p] = qktp.tile([P, S], BF16, tag="kt", name=f"kt{p}")
            ps = ps_w.tile([P, 512], F32, tag="w", name="pk")
            for mm in range(MC):
                nc.tensor.matmul(
                    ps[:], wk_t[mm][:, p * P:(p + 1) * P],
                    xkT[mm][:, nh * 512:(nh + 1) * 512],
                    start=(mm == 0), stop=(mm == MC - 1))
            nc.vector.tensor_scalar_add(
                KT[p][:, nh * 512:(nh + 1) * 512], ps[:], bk_t[:, p:p + 1])

        def mk_qk_items(p):
            items = []
            for nh in range(SQ // 512):
                items.append(lambda nh=nh: emit_qproj_nh(p, nh))
            for nh in range(S // 512):
                items.append(lambda nh=nh: emit_kproj_nh(p, nh))
            return items

        def mk_vgroup_item(stg, g):
            def f():
                xbs = []
                for j in range(4):
                    r = g * 4 + j
                    xin = stg.tile([P, D], F32, tag="xin", bufs=4, name="xin")
                    nc.sync.dma_start(xin[:], v_in.ap()[r * P:(r + 1) * P, :])
                    xbs.append(xin)
                vts = [stg.tile([P, 512], BF16, tag=f"vt{mm}", bufs=1,
                                name=f"vt{mm}") for mm in range(MC)]
                for mm in range(MC):
                    pst = ps_w.tile([P, 512], F32, tag="w", name="pst")
                    for j in range(4):
                        nc.tensor.transpose(
                            pst[:, j * P:(j + 1) * P],
                            xbs[j][:, mm * P:(mm + 1) * P], ident_cell[0])
                    nc.vector.tensor_copy(vts[mm][:], pst[:])
                cur_valT[g] = vts
            return f

        def mk_vproj_item(sc, nh):
            def f():
                g, j = sc // 4, sc % 4
                vts = cur_valT[g]
                vx = V[sc].rearrange("p (h c) -> p h c", c=65)
                if nh == 0:
                    nc.vector.tensor_copy(
                        vx[:, :, 64:65],
                        ones16[:].rearrange("p (h c) -> p h c", c=1))
                ps = ps_w.tile([P, 512], F32, tag="w", name="pvps")
                for mm in range(MC):
                    nc.tensor.matmul(
                        ps[:], vts[mm][:, j * P:(j + 1) * P],
                        wv_t[mm][:, nh * 512:(nh + 1) * 512],
                        start=(mm == 0), stop=(mm == MC - 1))
                nc.vector.tensor_copy(
                    vx[:, 8 * nh:8 * nh + 8, 0:64],
                    ps[:].rearrange("p (h c) -> p h c", c=64))
                if nh == 1:
                    vstate["ready"] = sc
            return f

        def normalize(pair, qs, pv1, pv2):
            """Evict PV psums wholesale (frees the banks fast), then
            softmax-normalize both heads (row 64 = sums)."""
            for hh, pvp in ((0, pv1), (1, pv2)):
                pb = normp.tile([65, 512], F32, tag="pb", bufs=2, name="pb")
                nc.vector.tensor_copy(pb[:], pvp[0:65, :])
                r0 = normp.tile([1, 512], F32, tag="r0", bufs=1, name="r0")
                nc.gpsimd.tensor_copy(r0[:], pb[64:65, :])
                rr = normp.tile([1, 512], F32, tag="rr", bufs=1, name="rr")
                nc.vector.reciprocal_approx_fast(rr[:], r0[:])
                bc = normp.tile([64, 512], F32, tag="bc", bufs=2, name="bc")
                nc.gpsimd.partition_broadcast(bc[:], rr[:])
                if hh == 0:
                    osl = OT[pair][0:64, qs]
                    nc.vector.tensor_mul(osl, pb[0:64, :], bc[:])
                    nc.vector.tensor_scalar_add(
                        osl, osl, bv_t[0:64, pair:pair + 1])
                else:
                    tmp = normp.tile([64, 512], BF16, tag="tmp", bufs=2,
                                     name="tmp")
                    nc.vector.tensor_mul(tmp[:], pb[0:64, :], bc[:])
                    osl = OT[pair][64:128, qs]
                    nc.sync.dma_start(osl, tmp[:])
                    nc.vector.tensor_scalar_add(
                        osl, osl, bv_t[64:128, pair:pair + 1])

        def emit_final_item(nh, sc):
            ns = slice(nh * 512, (nh + 1) * 512)
            ss = slice(sc * P, (sc + 1) * P)
            fps = ps_w.tile([P, 512], F32, tag="w", name="fps")
            for dk in range(DKC):
                nc.tensor.matmul(fps[:], OT[dk][:, ss], wo_t[dk][:, ns],
                                 start=(dk == 0), stop=False)
            nc.tensor.matmul(fps[:], ones_row_cell[0][:], wo_t[8][:, ns],
                             start=False, stop=True)
            ob = final_pool[0].tile([P, 512], F32, tag="ob", bufs=2, name="ob")
            nc.vector.tensor_copy(ob[:], fps[:])
            nc.sync.dma_start(out_d.ap()[ss, ns], ob[:])

        final_pool = [None]
        vitems = deque()
        kqstart = deque()
        qkitems = deque()
        woitems = deque()
        lateitems = deque()
        slot_ctr = [0]

        def pump_opportunistic():
            slot_ctr[0] += 1
            if kqstart:
                kqstart.popleft()()
            elif vitems:
                vitems.popleft()()
            elif qkitems:
                qkitems.popleft()()
            elif woitems:
                woitems.popleft()()
            elif lateitems:
                lateitems.popleft()()

        def emit_pv(pair, k2, e1, e2, pv1, pv2):
            c1 = (2 * pair) * 65
            c2 = (2 * pair + 1) * 65
            first = k2 == 0
            last = k2 == KC // 2 - 1
            nc.tensor.matmul(pv1[0:65, :], V[2 * k2][:, c1:c1 + 65],
                             e1[:, 0:512], start=first, stop=False)
            nc.tensor.matmul(pv2[0:65, :], V[2 * k2][:, c2:c2 + 65],
                             e2[:, 0:512], start=first, stop=False)
            nc.tensor.matmul(pv1[0:65, :], V[2 * k2 + 1][:, c1:c1 + 65],
                             e1[:, 512:1024], start=False, stop=last)
            nc.tensor.matmul(pv2[0:65, :], V[2 * k2 + 1][:, c2:c2 + 65],
                             e2[:, 512:1024], start=False, stop=last)

        def attn_half(p, qt):
                if p == 0 and qt == 1:
                    while kqstart:
                        kqstart.popleft()()
                qs = slice(qt * 512, (qt + 1) * 512)
                pv1 = ps_pv.tile([P, 512], F32, tag="pv", name="pv1")
                pv2 = ps_pv.tile([P, 512], F32, tag="pv", name="pv2")
                pend = None
                for k2 in range(KC // 2):
                    if p == 0:
                        while kqstart and kstate["ready"] < k2 // 2 + 1:
                            kqstart.popleft()()
                    ka = slice(2 * k2 * P, (2 * k2 + 1) * P)
                    kb = slice((2 * k2 + 1) * P, (2 * k2 + 2) * P)
                    s1 = ps_sc.tile([P, 1024], F32, tag="sc", name="s1")
                    s2 = ps_sc.tile([P, 1024], F32, tag="sc", name="s2")
                    nc.tensor.matmul(s1[:, 0:512], KT[p][0:64, ka],
                                     QT[p][0:64, qs], start=True, stop=True,
                                     tile_position=(0, 0))
                    nc.tensor.matmul(s2[:, 0:512], KT[p][64:128, ka],
                                     QT[p][64:128, qs], start=True, stop=True,
                                     tile_position=(64, 0))
                    nc.tensor.matmul(s1[:, 512:1024], KT[p][0:64, kb],
                                     QT[p][0:64, qs], start=True, stop=True,
                                     tile_position=(0, 0))
                    nc.tensor.matmul(s2[:, 512:1024], KT[p][64:128, kb],
                                     QT[p][64:128, qs], start=True, stop=True,
                                     tile_position=(64, 0))
                    e1 = ep.tile([P, 1024], BF16, tag="ea", name="e1")
                    e2 = ep.tile([P, 1024], BF16, tag="eb", name="e2")
                    nc.scalar.activation(e1[:], s1[:], EXP, scale=SCALE)
                    nc.scalar.activation(e2[:], s2[:], EXP, scale=SCALE)
                    if pend is not None:
                        while vitems and vstate["ready"] < 2 * pend[0] + 1:
                            vitems.popleft()()
                    pump_opportunistic()
                    if pend is not None:
                        emit_pv(p, pend[0], pend[1], pend[2], pv1, pv2)
                    pend = (k2, e1, e2)
                while vitems and vstate["ready"] < KC - 1:
                    vitems.popleft()()
                emit_pv(p, pend[0], pend[1], pend[2], pv1, pv2)
                normalize(p, qs, pv1, pv2)

        def attn_pair(p):
            attn_half(p, 0)
            attn_half(p, 1)

        # ---------- emission ----------
        with tc.tile_pool(name="stg", bufs=1) as stg:
            identf = stg.tile([P, P], F32, tag="identf", name="identf")
            make_identity(nc, identf[:])
            ident_cell[0] = identf[:]
            # pipelined startup: Q group 0 + Wq + Qproj(0,nh0); K group 0 +
            # Wk + Kproj(0,nh0); the rest of the Q/K pipeline rides the
            # pair-0 attention slots as need-driven items.
            emit_tgroup(stg, q_in, 0, xqT)
            for mm in range(MC):
                wq_t.append(emit_wcast(stg, wq_d, mm, wqkp, "wq"))
            emit_qproj_nh(0, 0)
            emit_tgroup(stg, k_in, 0, xkT)
            for mm in range(MC):
                wk_t.append(emit_wcast(stg, wk_d, mm, wqkp, "wk"))
            emit_kproj_nh(0, 0)
            kstate["ready"] = 1

            def mk_kstart_item(g):
                def f():
                    emit_tgroup(stg, k_in, g, xkT)
                    emit_kproj_nh(0, g)
                    kstate["ready"] = g + 1
                return f

            def qg1_item():
                emit_tgroup(stg, q_in, 1, xqT)
                emit_qproj_nh(0, 1)

            kqstart.append(mk_kstart_item(1))
            kqstart.append(qg1_item)
            kqstart.append(mk_kstart_item(2))
            kqstart.append(mk_kstart_item(3))

            def mk_wv_item(mm):
                def f():
                    wv_t.append(emit_wcast(stg, wv_d, mm, stg, "wv"))
                return f
            for mm in range(MC):
                vitems.append(mk_wv_item(mm))
            for g in range(S // 512):
                vitems.append(mk_vgroup_item(stg, g))
                for j in range(4):
                    for nh in range(2):
                        vitems.append(mk_vproj_item(4 * g + j, nh))

            qkitems.extend(mk_qk_items(1))
            attn_pair(0)
            while kqstart:
                kqstart.popleft()()
            while vitems:
                vitems.popleft()()
            while qkitems:
                qkitems.popleft()()
            qkitems.extend(mk_qk_items(2))
            attn_pair(1)
            while qkitems:
                qkitems.popleft()()

        with tc.tile_pool(name="wop", bufs=1) as wop:
            final_pool[0] = wop

            def mk_wo_item(mm):
                def f():
                    raw = wop.tile([P, D], F32, tag="woraw", bufs=2,
                                   name="woraw")
                    nc.sync.dma_start(raw[:], wo_d.ap()[mm * P:(mm + 1) * P, :])
                    t = wop.tile([P, D], BF16, tag=f"wo{mm}", name=f"wo{mm}")
                    nc.vector.tensor_copy(t[:], raw[:])
                    wo_t.append(t)
                return f
            for mm in range(MC):
                woitems.append(mk_wo_item(mm))

            def bo_item():
                bo_f = wop.tile([1, D], F32, tag="bof", name="bo_f")
                nc.sync.dma_start(bo_f[:], bo_d.ap().unsqueeze(0))
                bo_t = wop.tile([1, D], F32R, tag="bot", name="bo_t")
                nc.vector.tensor_copy(bo_t[:], bo_f[:])
                wo_t.append(bo_t)  # sentinel slot 8
                onesr_f = wop.tile([1, P], F32, tag="onesf", name="onesr_f")
                nc.vector.memset(onesr_f[:], 1.0)
                ones_row = wop.tile([1, P], F32R, tag="ones", name="ones_row")
                nc.vector.tensor_copy(ones_row[:], onesr_f[:])
                ones_row_cell[0] = ones_row
            woitems.append(bo_item)

            for p in range(2, DKC - 1):
                qkitems.extend(mk_qk_items(p + 1))
                attn_pair(p)
                while qkitems:
                    qkitems.popleft()()
            while woitems:
                woitems.popleft()()
            attn_half(DKC - 1, 0)
            # first-half (qt0) final projection rides pair-7 qt1
            for nh in range(2):
                for sc in range(4):
                    lateitems.append(
                        lambda nh=nh, sc=sc: emit_final_item(nh, sc))
            attn_half(DKC - 1, 1)
            while woitems:
                woitems.popleft()()
            while lateitems:
                lateitems.popleft()()
            for nh in range(2):
                for sc in range(4, SQ // P):
                    emit_final_item(nh, sc)


def get_nc():
    global _CACHED_NC
    if _CACHED_NC is None:
        _CACHED_NC = build_nc()
    return _CACHED_NC


def run(inputs, **kwargs):
    """Run on 8 cores; returns (full_output, BassKernelResults)."""
    nc = get_nc()
    queries = np.ascontiguousarray(np.asarray(inputs["queries"], np.float32))
    keys = np.ascontiguousarray(np.asarray(inputs["keys"], np.float32))
    values = np.ascontiguousarray(np.asarray(inputs["values"], np.float32))
    base = {
        "wq": np.ascontiguousarray(np.asarray(inputs["Wq"], np.float32)),
        "wk": np.ascontiguousarray(np.asarray(inputs["Wk"], np.float32)),
        "wv": np.ascontiguousarray(np.asarray(inputs["Wv"], np.float32)),
        "wo": np.ascontiguousarray(np.asarray(inputs["Wo"], np.float32)),
        "bq": np.ascontiguousarray(np.asarray(inputs["bq"], np.float32)),
        "bk": np.ascontiguousarray(np.asarray(inputs["bk"], np.float32)),
        "bv": np.ascontiguousarray(np.asarray(inputs["bv"], np.float32)),
        "bo": np.ascontiguousarray(np.asarray(inputs["bo"], np.float32)),
    }
    in_maps = []
    for c in range(N_CORES):
        b, qh = c // 2, c % 2
        m = dict(base)
        m["q_in"] = np.ascontiguousarray(queries[b, qh * SQ:(qh + 1) * SQ])
        m["k_in"] = keys[b]
        m["v_in"] = values[b]
        in_maps.append(m)
    res = bass_utils.run_bass_kernel_spmd(
        nc, in_maps, core_ids=list(range(N_CORES)), **kwargs)
    out = np.empty((B, S, D), np.float32)
    for c in range(N_CORES):
        b, qh = c // 2, c % 2
        out[b, qh * SQ:(qh + 1) * SQ] = res.results[c]["out"]
    return out, res


def kernel(**inputs):
    out, _ = run(inputs)
    return out


if __name__ == "__main__":
    rng = np.random.default_rng(0)
    ins = {
        "queries": rng.standard_normal((B, S, D), dtype=np.float32),
        "keys": rng.standard_normal((B, S, D), dtype=np.float32),
        "values": rng.standard_normal((B, S, D), dtype=np.float32),
        "Wq": (rng.standard_normal((D, D), dtype=np.float32) / 32),
        "bq": np.zeros(D, np.float32),
        "Wk": (rng.standard_normal((D, D), dtype=np.float32) / 32),
        "bk": np.zeros(D, np.float32),
        "Wv": (rng.standard_normal((D, D), dtype=np.float32) / 32),
        "bv": np.zeros(D, np.float32),
        "Wo": (rng.standard_normal((D, D), dtype=np.float32) / 32),
        "bo": np.zeros(D, np.float32),
    }
    out = kernel(**ins)
    print("out", out.shape, out.dtype, np.abs(out).mean())
